# revision 1
# baseline (speedup 1.0000x reference)
"""NNConv message-passing GNN on 8 Trainium2 NeuronCores.

Strategy: the reference materializes per-edge weight matrices We [E,32,32]
(512MB) and reads them every layer. We is rank-8 (built from eh [E,8]),
so instead we compute, per edge tile of 128 edges:
    T9 = h_srcT.T @ K2r9        # [128, 288]  (o,c)-major, c in 0..8, c=8 is bias
    P9 = T9 * eh9 (broadcast)   # DVE
    msg = sum_c P9              # DVE grouped reduce -> [128, 32]
    aggT[:, win] += msg.T @ S   # PE matmul against host-built scatter matrix
Everything stays in SBUF; HBM traffic is only inputs + the per-layer
AllGather of h slices.

Sharding: edges sorted by dst, core k owns dst in [2048k, 2048(k+1)).
Fixed structure: 32 windows x 64 nodes, 5 edge-tiles (640 slots) per window,
160 tiles = 20480 edge slots per core (padding slots have S=0, eh=0).
Per layer each core computes h_new for its 2048-node slice, then AllGather.
"""

import sys

import numpy as np

sys.path.insert(0, "/opt/trn_rl_repo")

import concourse.bass as bass  # noqa: E402
import concourse.tile as tile  # noqa: E402
from concourse import bacc, mybir  # noqa: E402
from concourse.bass_utils import run_bass_kernel_spmd  # noqa: E402

N = 16384
E = 131072
W = 32
DEPTH = 4
NCORES = 8
NPC = N // NCORES          # 2048 nodes per core
WIN = 64                   # nodes per scatter window
TPW = 5                    # edge tiles per window (640 slots)
WPC = NPC // WIN           # 32 windows per core
NT = WPC * TPW             # 160 edge tiles per core
EPC = NT * 128             # 20480 edge slots per core
GRP = 512                  # nodes per psum accumulation group
NGRP = NPC // GRP          # 4
F32 = mybir.dt.float32
I16 = mybir.dt.int16
AF = mybir.ActivationFunctionType
ALU = mybir.AluOpType


def build_program(fc2b_val=0.0, repeat=1, fake_collective=False):
    nc = bacc.Bacc("TRN2", target_bir_lowering=False, debug=False,
                   num_devices=NCORES)

    d_xt = nc.declare_dram_parameter("xt", [4, N], F32, isOutput=False)
    d_fc1 = nc.declare_dram_parameter("fc1a", [4, W], F32, isOutput=False)
    d_k1 = nc.declare_dram_parameter("k1a", [8, 8], F32, isOutput=False)
    d_k2 = nc.declare_dram_parameter("k2r", [W, 288], F32, isOutput=False)
    d_root = nc.declare_dram_parameter("rootw", [W, W], F32, isOutput=False)
    d_cb = nc.declare_dram_parameter("cbias", [W, 1], F32, isOutput=False)
    d_fc2 = nc.declare_dram_parameter("fc2w", [W, 1], F32, isOutput=False)
    d_eat = nc.declare_dram_parameter("eat", [8, EPC], F32, isOutput=False)
    d_gidx = nc.declare_dram_parameter("gidx", [32, EPC // 16], I16, isOutput=False)
    d_oidx = nc.declare_dram_parameter("oidx", [32, NPC // 16], I16, isOutput=False)
    d_s = nc.declare_dram_parameter("stil", [128, NT * WIN], F32, isOutput=False)
    d_out = nc.declare_dram_parameter("out", [1, NPC], F32, isOutput=True)

    cc_in = nc.dram_tensor("cc_in", [W, NPC], F32)
    cc_out = nc.dram_tensor("cc_out", [NCORES * W, NPC], F32, addr_space="Shared")

    from contextlib import ExitStack

    with ExitStack() as ctx:
        tc = ctx.enter_context(tile.TileContext(nc))

        persist = ctx.enter_context(tc.tile_pool(name="persist", bufs=1))
        hT = persist.tile([W, N], F32, tag="hT")
        hown = persist.tile([W, NPC], F32, tag="hown")
        eh9 = persist.tile([128, NT * 9], F32, tag="eh9")
        s_sb = persist.tile([128, NT * WIN], F32, tag="s_sb")
        gidx = persist.tile([32, EPC // 16], I16, tag="gidx")
        oidx = persist.tile([32, NPC // 16], I16, tag="oidx")
        k2r = persist.tile([W, 288], F32, tag="k2r")
        fc1a = persist.tile([4, W], F32, tag="fc1a")
        k1a = persist.tile([8, 8], F32, tag="k1a")
        rootw = persist.tile([W, W], F32, tag="rootw")
        cbias = persist.tile([W, 1], F32, tag="cbias")
        fc2w = persist.tile([W, 1], F32, tag="fc2w")
        outT = persist.tile([1, NPC], F32, tag="outT")

        stage = ctx.enter_context(tc.tile_pool(name="stage", bufs=2))
        gpool = ctx.enter_context(tc.tile_pool(name="gth", bufs=3))
        vpool = ctx.enter_context(tc.tile_pool(name="vec", bufs=3))
        mpool = ctx.enter_context(tc.tile_pool(name="msg", bufs=3))
        ps_t9 = ctx.enter_context(tc.tile_pool(name="t9", bufs=3, space="PSUM"))
        ps_ag = ctx.enter_context(tc.tile_pool(name="agg", bufs=2, space="PSUM"))
        ps_sm = ctx.enter_context(tc.tile_pool(name="small", bufs=2, space="PSUM"))

        dma = nc.sync.dma_start

        dma(k2r[:], d_k2[:])
        dma(fc1a[:], d_fc1[:])
        dma(k1a[:], d_k1[:])
        dma(rootw[:], d_root[:])
        dma(cbias[:], d_cb[:])
        dma(fc2w[:], d_fc2[:])
        dma(gidx[:], d_gidx[:])
        dma(oidx[:], d_oidx[:])
        qs = NT * WIN // 4
        for i in range(4):
            dma(s_sb[:, i * qs:(i + 1) * qs], d_s[:, i * qs:(i + 1) * qs])
        nc.vector.memset(eh9[:], 1.0)

        # node encoder: hT = (x @ fc1_w.T + fc1_b).T, computed fully on every core
        for ch in range(8):
            xt_t = stage.tile([4, 2048], F32, tag="stg")
            dma(xt_t[:], d_xt[:, ch * 2048:(ch + 1) * 2048])
            for q in range(4):
                ps = ps_sm.tile([W, GRP], F32, tag="sm")
                nc.tensor.matmul(ps[:], lhsT=fc1a[:], rhs=xt_t[:, q * GRP:(q + 1) * GRP],
                                 start=True, stop=True)
                col = ch * 2048 + q * GRP
                if q % 2 == 0:
                    nc.scalar.activation(hT[:, col:col + GRP], ps[:], AF.Copy)
                else:
                    nc.vector.tensor_copy(hT[:, col:col + GRP], ps[:])

        nc.gpsimd.ap_gather(hown[:], hT[:], oidx[:],
                            channels=32, num_elems=N, d=1, num_idxs=NPC)

        # edge network first layer: eh9[:, 9t:9t+8] = relu(ea_aug.T @ k1a), col 8 stays 1.0
        for ch in range(10):
            ea_t = stage.tile([8, 2048], F32, tag="stg")
            dma(ea_t[:], d_eat[:, ch * 2048:(ch + 1) * 2048])
            for j in range(16):
                t = ch * 16 + j
                ps = ps_sm.tile([128, 8], F32, tag="sm")
                nc.tensor.matmul(ps[:], lhsT=ea_t[:, j * 128:(j + 1) * 128], rhs=k1a[:],
                                 start=True, stop=True)
                nc.scalar.activation(eh9[:, t * 9:t * 9 + 8], ps[:], AF.Relu)

        gth = None
        for layer in range(DEPTH * repeat):
            for g in range(NGRP):
                agg = ps_ag.tile([W, GRP], F32)
                nc.tensor.matmul(agg[:], lhsT=rootw[:], rhs=hown[:, g * GRP:(g + 1) * GRP],
                                 start=True, stop=False)
                for w8 in range(WPC // NGRP):
                    for t5 in range(TPW):
                        t = (g * (WPC // NGRP) + w8) * TPW + t5
                        if t % 4 == 0:
                            gth = gpool.tile([32, 512], F32, tag="gth")
                            c = t // 4
                            nc.gpsimd.ap_gather(gth[:], hT[:], gidx[:, c * 32:(c + 1) * 32],
                                                channels=32, num_elems=N, d=1, num_idxs=512)
                        t9 = ps_t9.tile([128, 288], F32)
                        j = t % 4
                        nc.tensor.matmul(t9[:], lhsT=gth[:, j * 128:(j + 1) * 128], rhs=k2r[:],
                                         start=True, stop=True)
                        p9 = vpool.tile([128, 288], F32, tag="p9")
                        ehv = (eh9[:, t * 9:(t + 1) * 9]
                               .rearrange('p (x c) -> p x c', x=1)
                               .broadcast_to([128, W, 9]))
                        nc.vector.tensor_tensor(
                            p9[:].rearrange('p (o c) -> p o c', c=9),
                            t9[:].rearrange('p (o c) -> p o c', c=9),
                            ehv, op=ALU.mult)
                        msg = mpool.tile([128, W], F32, tag="msg")
                        nc.vector.reduce_sum(msg[:], p9[:].rearrange('p (o c) -> p o c', c=9),
                                             axis=mybir.AxisListType.X)
                        nc.tensor.matmul(agg[:, w8 * WIN:(w8 + 1) * WIN],
                                         lhsT=msg[:], rhs=s_sb[:, t * WIN:(t + 1) * WIN],
                                         start=False,
                                         stop=(w8 == WPC // NGRP - 1 and t5 == TPW - 1))
                nc.scalar.activation(hown[:, g * GRP:(g + 1) * GRP], agg[:], AF.Relu,
                                     bias=cbias[:])
            if layer % DEPTH != DEPTH - 1:
                for i in range(4):
                    dma(cc_in[:, i * GRP:(i + 1) * GRP], hown[:, i * GRP:(i + 1) * GRP])
                if fake_collective:
                    dma(cc_out[:W, :], cc_in[:, :])
                else:
                    nc.gpsimd.collective_compute(
                        "AllGather", ALU.bypass,
                        replica_groups=[list(range(NCORES))],
                        ins=[cc_in[:].opt()],
                        outs=[cc_out[:].opt()],
                    )
                for r in range(NCORES):
                    dma(hT[:, r * NPC:(r + 1) * NPC], cc_out[r * W:(r + 1) * W, :])

        for q in range(NGRP):
            ps = ps_sm.tile([1, GRP], F32, tag="sm")
            nc.tensor.matmul(ps[:], lhsT=fc2w[:], rhs=hown[:, q * GRP:(q + 1) * GRP],
                             start=True, stop=True)
            nc.scalar.activation(outT[:, q * GRP:(q + 1) * GRP], ps[:], AF.Copy,
                                 bias=float(fc2b_val))
        dma(d_out[:, :], outT[:])

    nc.finalize()
    return nc


def _prep_inputs(inputs):
    x = np.asarray(inputs["x"], np.float32)
    ei = np.asarray(inputs["edge_index"])
    ea = np.asarray(inputs["edge_attr"], np.float32)
    fc1_w = np.asarray(inputs["fc1_w"], np.float32)
    fc1_b = np.asarray(inputs["fc1_b"], np.float32)
    k1_w = np.asarray(inputs["k1_w"], np.float32)
    k1_b = np.asarray(inputs["k1_b"], np.float32)
    k2_w = np.asarray(inputs["k2_w"], np.float32)
    k2_b = np.asarray(inputs["k2_b"], np.float32)
    root_w = np.asarray(inputs["root_w"], np.float32)
    conv_bias = np.asarray(inputs["conv_bias"], np.float32)
    fc2_w = np.asarray(inputs["fc2_w"], np.float32)
    fc2_b = np.asarray(inputs["fc2_b"], np.float32)

    src = ei[0].astype(np.int64)
    dst = ei[1].astype(np.int64)
    deg = np.clip(np.bincount(dst, minlength=N).astype(np.float32), 1.0, None)

    xt = np.concatenate([x.T, np.ones((1, N), np.float32)], axis=0)
    fc1a = np.concatenate([fc1_w.T, fc1_b[None, :]], axis=0)
    k1a = np.concatenate([k1_w.T, k1_b[None, :]], axis=0)
    k2r = np.concatenate(
        [k2_w.reshape(W, W, 8), k2_b.reshape(W, W, 1)], axis=2
    ).reshape(W, 288)
    shared = {
        "xt": np.ascontiguousarray(xt),
        "fc1a": np.ascontiguousarray(fc1a),
        "k1a": np.ascontiguousarray(k1a),
        "k2r": np.ascontiguousarray(k2r),
        "rootw": np.ascontiguousarray(root_w),
        "cbias": np.ascontiguousarray(conv_bias[:, None]),
        "fc2w": np.ascontiguousarray(fc2_w.T),
    }
    shared["_fc2b_val"] = float(fc2_b[0])

    in_maps = []
    jarr = np.arange(EPC)
    for k in range(NCORES):
        m = (dst >= k * NPC) & (dst < (k + 1) * NPC)
        es, ed, eak = src[m], dst[m], ea[m]
        local = ed - k * NPC
        win = local // WIN
        jloc = local % WIN

        gsrc = np.zeros(EPC, np.int64)
        sval = np.zeros(EPC, np.float32)
        jl = np.zeros(EPC, np.int64)
        eat = np.zeros((8, EPC), np.float32)
        for w in range(WPC):
            idxs = np.flatnonzero(win == w)
            assert len(idxs) <= TPW * 128, f"window overflow: {len(idxs)}"
            base = w * TPW * 128
            sl = slice(base, base + len(idxs))
            gsrc[sl] = es[idxs]
            sval[sl] = 1.0 / deg[ed[idxs]]
            jl[sl] = jloc[idxs]
            eat[:7, sl] = eak[idxs].T
            eat[7, sl] = 1.0

        S = np.zeros((NT, 128, WIN), np.float32)
        S[jarr // 128, jarr % 128, jl] = sval
        stil = np.ascontiguousarray(S.transpose(1, 0, 2).reshape(128, NT * WIN))

        gx = np.zeros((32, EPC // 16), np.int16)
        gx[jarr % 16, jarr // 16] = gsrc
        gx[16 + jarr % 16, jarr // 16] = gsrc

        oj = np.arange(NPC)
        ox = np.zeros((32, NPC // 16), np.int16)
        ox[oj % 16, oj // 16] = k * NPC + oj
        ox[16 + oj % 16, oj // 16] = k * NPC + oj

        in_maps.append({
            **shared,
            "eat": eat,
            "gidx": gx,
            "oidx": ox,
            "stil": stil,
        })
    return in_maps


def _run(inputs, trace=False, **kw):
    in_maps = _prep_inputs(inputs)
    fc2b_val = in_maps[0].pop("_fc2b_val")
    for m in in_maps[1:]:
        m.pop("_fc2b_val")
    nc = build_program(fc2b_val)
    res = run_bass_kernel_spmd(nc, in_maps, core_ids=list(range(NCORES)),
                               trace=trace, **kw)
    outs = [res.results[k]["out"].reshape(1, NPC) for k in range(NCORES)]
    full = np.concatenate(outs, axis=1).T.astype(np.float32)
    return full, res


def kernel(**inputs) -> np.ndarray:
    return _run(inputs, trace=False)[0]



# revision 2
# speedup vs baseline: 2.3586x; 2.3586x over previous
"""NNConv message-passing GNN on 8 Trainium2 NeuronCores — v2 (edge-major, bf16).

Scheme ("H9-first"): the per-edge weight matrix We [E,32,32] is rank-9 in
channel space: We_e = sum_c eh9[e,c] * M_c with 9 shared 32x32 matrices M_c
(8 from k2_w + 1 bias). So

    msg_e[o] = sum_c eh9[e,c] * H9[src_e, (o,c)],   H9[n,(o,c)] = h[n,:] @ M_c

H9 is computed per NODE (16 matmuls/core/layer), AllGathered as a bf16 DRAM
table [N, 384] (288 payload + pad for the 256B-stride dma_gather rule), and
gathered per edge in edge-major layout [128e, 384] tiles via dma_gather.
Per edge tile: one DVE broadcast-multiply P = gat * eh9 (bf16), one scatter
matmul UT[64,288] += S_t.T @ P per tile against the host-built scatter matrix
(1/deg folded in), a per-window DVE reduce over c, a transpose matmul back to
[o, n] layout, root matmul + bias + relu.  Everything bf16 except PSUM
accumulation and the final output.

Sharding: edges sorted by dst; core k owns dst in [2048k, 2048(k+1)).
32 windows x 64 nodes, 5 edge-tiles (640 slots) per window, 160 tiles
= 20480 edge slots per core.  Per layer: AllGather of the per-core H9 slice.
The edge-network (eh) is computed on host (it is layer-invariant).
"""

import sys
from contextlib import ExitStack

import numpy as np
import ml_dtypes

sys.path.insert(0, "/opt/trn_rl_repo")

import concourse.bass as bass  # noqa: E402
import concourse.tile as tile  # noqa: E402
from concourse import bacc, mybir  # noqa: E402
from concourse.bass_utils import run_bass_kernel_spmd  # noqa: E402

N = 16384
E = 131072
W = 32
DEPTH = 4
NCORES = 8
NPC = N // NCORES          # 2048 nodes per core
WIN = 64                   # nodes per scatter window
TPW = 5                    # edge tiles per window (640 slots)
WPC = NPC // WIN           # 32 windows per core
NT = WPC * TPW             # 160 edge tiles per core
EPC = NT * 128             # 20480 edge slots per core
GRP = 512                  # nodes per group
NGRP = NPC // GRP          # 4
TPG = NT // NGRP           # 40 tiles per group
EPG = TPG * 128            # 5120 edge slots per group
H9W = W * 9                # 288 payload columns of H9
STEP = 384                 # padded H9 row (768 bytes, 256B-aligned)
F32 = mybir.dt.float32
BF16 = mybir.dt.bfloat16
I16 = mybir.dt.int16
AF = mybir.ActivationFunctionType
ALU = mybir.AluOpType
BF = ml_dtypes.bfloat16


def build_program(fc2b_val=0.0, fake_collective=False):
    nc = bacc.Bacc("TRN2", target_bir_lowering=False, debug=False,
                   num_devices=NCORES)

    d_xtk = nc.declare_dram_parameter("xtk", [4, NPC], BF16, isOutput=False)
    d_fc1 = nc.declare_dram_parameter("fc1a", [4, W], BF16, isOutput=False)
    d_k2 = nc.declare_dram_parameter("k2pr", [W, H9W], BF16, isOutput=False)
    d_root = nc.declare_dram_parameter("rootw", [W, W], BF16, isOutput=False)
    d_cb = nc.declare_dram_parameter("cbias", [W, 1], F32, isOutput=False)
    d_fc2 = nc.declare_dram_parameter("fc2w", [W, 1], BF16, isOutput=False)
    d_eh = nc.declare_dram_parameter("eh9b", [128, NT * 9], BF16, isOutput=False)
    d_s = nc.declare_dram_parameter("stil", [128, NT * WIN], BF16, isOutput=False)
    d_gidx = nc.declare_dram_parameter("gidx", [128, EPC // 16], I16, isOutput=False)
    d_id = nc.declare_dram_parameter("ident", [128, 128], BF16, isOutput=False)
    d_out = nc.declare_dram_parameter("out", [1, NPC], F32, isOutput=True)

    cc_in = nc.dram_tensor("cc_in", [NPC, STEP], BF16)
    cc_out = nc.dram_tensor("cc_out", [N, STEP], BF16, addr_space="Shared")

    with ExitStack() as ctx:
        tc = ctx.enter_context(tile.TileContext(nc))

        persist = ctx.enter_context(tc.tile_pool(name="persist", bufs=1))
        hown = persist.tile([W, NPC], BF16, tag="hown")
        s_sb = persist.tile([128, NT * WIN], BF16, tag="s_sb")
        eh9b = persist.tile([128, NT * 9], BF16, tag="eh9b")
        gidx = persist.tile([128, EPC // 16], I16, tag="gidx")
        k2pr = persist.tile([W, H9W], BF16, tag="k2pr")
        fc1a = persist.tile([4, W], BF16, tag="fc1a")
        rootw = persist.tile([W, W], BF16, tag="rootw")
        cbias = persist.tile([W, 1], F32, tag="cbias")
        fc2w = persist.tile([W, 1], BF16, tag="fc2w")
        ident = persist.tile([128, 128], BF16, tag="ident")
        outT = persist.tile([1, NPC], F32, tag="outT")

        stage = ctx.enter_context(tc.tile_pool(name="stage", bufs=2))
        gpool = ctx.enter_context(tc.tile_pool(name="gat", bufs=2))
        ppool = ctx.enter_context(tc.tile_pool(name="p9", bufs=3))
        h9pool = ctx.enter_context(tc.tile_pool(name="h9sb", bufs=2))
        atpool = ctx.enter_context(tc.tile_pool(name="aggT", bufs=2))
        ps_ut = ctx.enter_context(tc.tile_pool(name="ut", bufs=2, space="PSUM"))
        ps_agg = ctx.enter_context(tc.tile_pool(name="agg", bufs=2, space="PSUM"))
        ps_h9 = ctx.enter_context(tc.tile_pool(name="h9", bufs=2, space="PSUM"))
        ps_sm = ctx.enter_context(tc.tile_pool(name="small", bufs=2, space="PSUM"))

        dma = nc.sync.dma_start

        dma(fc1a[:], d_fc1[:])
        dma(k2pr[:], d_k2[:])
        dma(rootw[:], d_root[:])
        dma(cbias[:], d_cb[:])
        dma(fc2w[:], d_fc2[:])
        dma(ident[:], d_id[:])
        dma(gidx[:], d_gidx[:])
        dma(eh9b[:], d_eh[:])
        hs = NT * WIN // 2
        for i in range(2):
            dma(s_sb[:, i * hs:(i + 1) * hs], d_s[:, i * hs:(i + 1) * hs])

        # node encoder for own slice: hown = (x @ fc1_w.T + fc1_b).T
        xtk = stage.tile([4, NPC], BF16, tag="stg")
        dma(xtk[:], d_xtk[:])
        for q in range(NGRP):
            ps = ps_sm.tile([W, GRP], F32, tag="sm")
            nc.tensor.matmul(ps[:], lhsT=fc1a[:], rhs=xtk[:, q * GRP:(q + 1) * GRP],
                             start=True, stop=True)
            nc.scalar.activation(hown[:, q * GRP:(q + 1) * GRP], ps[:], AF.Copy)

        for layer in range(DEPTH):
            # --- H9 for own nodes -> cc_in -> AllGather -> cc_out ---
            for g in range(NGRP):
                h9t = h9pool.tile([128, NGRP, H9W], BF16, tag="h9t")
                for m in range(NGRP):
                    nt = g * NGRP + m
                    ph = ps_h9.tile([128, H9W], F32)
                    nc.tensor.matmul(ph[:], lhsT=hown[:, nt * 128:(nt + 1) * 128],
                                     rhs=k2pr[:], start=True, stop=True)
                    nc.scalar.activation(h9t[:, m, :], ph[:], AF.Copy)
                dma(cc_in[g * GRP:(g + 1) * GRP, 0:H9W]
                    .rearrange('(t p) e -> p t e', p=128), h9t[:])
            if fake_collective:
                dma(cc_out[0:NPC, :], cc_in[:, :])
            else:
                nc.gpsimd.collective_compute(
                    "AllGather", ALU.bypass,
                    replica_groups=[list(range(NCORES))],
                    ins=[cc_in[:].opt()],
                    outs=[cc_out[:].opt()],
                )

            # --- per group: gather H9[src], per-tile multiply+scatter ---
            for g in range(NGRP):
                gat = gpool.tile([128, TPG, STEP], BF16, tag="gat")
                GCH = 512                       # idxs per dma_gather call
                for c in range(EPG // GCH):
                    ct = GCH // 128             # tiles per chunk
                    nc.gpsimd.dma_gather(
                        gat[:, c * ct:(c + 1) * ct, :], cc_out[:],
                        gidx[:, g * (EPG // 16) + c * (GCH // 16):
                             g * (EPG // 16) + (c + 1) * (GCH // 16)],
                        num_idxs=GCH, num_idxs_reg=GCH, elem_size=STEP,
                    )
                agg = ps_agg.tile([W, GRP], F32)
                nc.tensor.matmul(agg[:], lhsT=rootw[:],
                                 rhs=hown[:, g * GRP:(g + 1) * GRP],
                                 start=True, stop=False)
                atile = atpool.tile([128, NGRP, W], BF16, tag="aggT")
                for wg in range(WPC // NGRP):       # 8 windows in group
                    w = g * (WPC // NGRP) + wg
                    t0 = w * TPW
                    j0 = t0 - g * TPG
                    base = 64 * (wg % 2)
                    p9t = ppool.tile([128, TPW, H9W], BF16, tag="p9t")
                    ehv = (eh9b[:, t0 * 9:(t0 + TPW) * 9]
                           .rearrange('p (t c) -> p t c', c=9)
                           .rearrange('p t (x c) -> p t x c', x=1)
                           .broadcast_to([128, TPW, W, 9]))
                    nc.vector.tensor_tensor(
                        p9t[:].rearrange('p t (o c) -> p t o c', c=9),
                        gat[:, j0:j0 + TPW, 0:H9W].rearrange('p t (o c) -> p t o c', c=9),
                        ehv, op=ALU.mult)
                    ut = ps_ut.tile([128, H9W], F32)
                    for t5 in range(TPW):
                        t = t0 + t5
                        nc.tensor.matmul(
                            ut[base:base + 64, :],
                            lhsT=s_sb[:, t * WIN:(t + 1) * WIN],
                            rhs=p9t[:, t5, :],
                            start=(t5 == 0), stop=(t5 == TPW - 1),
                            tile_position=(0, base))
                    m = wg // 2
                    with nc.allow_low_precision(reason="bf16 agg feeds bf16 matmuls"):
                        nc.vector.reduce_sum(
                            atile[base:base + 64, m, :],
                            ut[base:base + 64, :].rearrange('p (o c) -> p o c', c=9),
                            axis=mybir.AxisListType.X)
                for m in range(NGRP):
                    nc.tensor.matmul(agg[:, m * 128:(m + 1) * 128],
                                     lhsT=atile[:, m, :], rhs=ident[:],
                                     start=False, stop=(m == NGRP - 1))
                nc.scalar.activation(hown[:, g * GRP:(g + 1) * GRP], agg[:],
                                     AF.Relu, bias=cbias[:])

        for q in range(NGRP):
            ps = ps_sm.tile([1, GRP], F32, tag="sm")
            nc.tensor.matmul(ps[:], lhsT=fc2w[:], rhs=hown[:, q * GRP:(q + 1) * GRP],
                             start=True, stop=True)
            nc.scalar.activation(outT[:, q * GRP:(q + 1) * GRP], ps[:], AF.Copy,
                                 bias=float(fc2b_val))
        dma(d_out[:, :], outT[:])

    nc.finalize()
    return nc


def _prep_inputs(inputs):
    x = np.asarray(inputs["x"], np.float32)
    ei = np.asarray(inputs["edge_index"])
    ea = np.asarray(inputs["edge_attr"], np.float32)
    fc1_w = np.asarray(inputs["fc1_w"], np.float32)
    fc1_b = np.asarray(inputs["fc1_b"], np.float32)
    k1_w = np.asarray(inputs["k1_w"], np.float32)
    k1_b = np.asarray(inputs["k1_b"], np.float32)
    k2_w = np.asarray(inputs["k2_w"], np.float32)
    k2_b = np.asarray(inputs["k2_b"], np.float32)
    root_w = np.asarray(inputs["root_w"], np.float32)
    conv_bias = np.asarray(inputs["conv_bias"], np.float32)
    fc2_w = np.asarray(inputs["fc2_w"], np.float32)
    fc2_b = np.asarray(inputs["fc2_b"], np.float32)

    src = ei[0].astype(np.int64)
    dst = ei[1].astype(np.int64)
    deg = np.clip(np.bincount(dst, minlength=N).astype(np.float32), 1.0, None)

    # edge network on host (layer-invariant): eh9 = [relu(ea @ k1.T + b), 1]
    eh = np.maximum(ea @ k1_w.T + k1_b, 0.0)            # [E, 8]
    eh9_full = np.concatenate([eh, np.ones((E, 1), np.float32)], axis=1)

    xt = np.concatenate([x.T, np.ones((1, N), np.float32)], axis=0)  # [4, N]
    fc1a = np.concatenate([fc1_w.T, fc1_b[None, :]], axis=0)
    # k2pr[i, o*9+c] = M_c[i, o]; M_c[i,o] = k2_w[i*W+o, c] (c<8), k2_b[i*W+o] (c=8)
    k2pr = np.concatenate(
        [k2_w.reshape(W, W, 8), k2_b.reshape(W, W, 1)], axis=2
    ).reshape(W, H9W)

    shared = {
        "fc1a": fc1a.astype(BF),
        "k2pr": k2pr.astype(BF),
        "rootw": root_w.astype(BF),
        "cbias": np.ascontiguousarray(conv_bias[:, None]).astype(np.float32),
        "fc2w": np.ascontiguousarray(fc2_w.T).astype(BF),
        "ident": np.eye(128, dtype=np.float32).astype(BF),
    }

    in_maps = []
    jarr = np.arange(EPC)
    for k in range(NCORES):
        m = (dst >= k * NPC) & (dst < (k + 1) * NPC)
        es, ed, ehk = src[m], dst[m], eh9_full[m]
        local = ed - k * NPC
        win = local // WIN
        jloc = local % WIN

        gsrc = np.zeros(EPC, np.int64)
        sval = np.zeros(EPC, np.float32)
        jl = np.zeros(EPC, np.int64)
        eh9s = np.zeros((EPC, 9), np.float32)
        for w in range(WPC):
            idxs = np.flatnonzero(win == w)
            assert len(idxs) <= TPW * 128, f"window overflow: {len(idxs)}"
            base = w * TPW * 128
            sl = slice(base, base + len(idxs))
            gsrc[sl] = es[idxs]
            sval[sl] = 1.0 / deg[ed[idxs]]
            jl[sl] = jloc[idxs]
            eh9s[sl] = ehk[idxs]

        S = np.zeros((NT, 128, WIN), np.float32)
        S[jarr // 128, jarr % 128, jl] = sval
        stil = np.ascontiguousarray(S.transpose(1, 0, 2).reshape(128, NT * WIN))

        eh9b = np.ascontiguousarray(
            eh9s.reshape(NT, 128, 9).transpose(1, 0, 2).reshape(128, NT * 9))

        gx = np.zeros((128, EPC // 16), np.int16)
        for g in range(NGRP):
            i = np.arange(EPG)
            v = gsrc[g * EPG:(g + 1) * EPG]
            for a in range(8):
                gx[16 * a + i % 16, g * (EPG // 16) + i // 16] = v

        in_maps.append({
            **shared,
            "xtk": np.ascontiguousarray(xt[:, k * NPC:(k + 1) * NPC]).astype(BF),
            "eh9b": eh9b.astype(BF),
            "stil": stil.astype(BF),
            "gidx": gx,
        })
    return in_maps


def _run(inputs, trace=False, fake_collective=False, **kw):
    in_maps = _prep_inputs(inputs)
    fc2_b = np.asarray(inputs["fc2_b"], np.float32)
    nc = build_program(float(fc2_b[0]), fake_collective=fake_collective)
    res = run_bass_kernel_spmd(nc, in_maps, core_ids=list(range(NCORES)),
                               trace=trace, **kw)
    outs = [res.results[k]["out"].reshape(1, NPC) for k in range(NCORES)]
    full = np.concatenate(outs, axis=1).T.astype(np.float32)
    return full, res


def kernel(**inputs) -> np.ndarray:
    return _run(inputs, trace=False)[0]


# revision 3
# speedup vs baseline: 2.4742x; 1.0490x over previous
"""NNConv message-passing GNN on 8 Trainium2 NeuronCores — v2 (edge-major, bf16).

Scheme ("H9-first"): the per-edge weight matrix We [E,32,32] is rank-9 in
channel space: We_e = sum_c eh9[e,c] * M_c with 9 shared 32x32 matrices M_c
(8 from k2_w + 1 bias). So

    msg_e[o] = sum_c eh9[e,c] * H9[src_e, (o,c)],   H9[n,(o,c)] = h[n,:] @ M_c

H9 is computed per NODE (16 matmuls/core/layer), AllGathered as a bf16 DRAM
table [N, 384] (288 payload + pad for the 256B-stride dma_gather rule), and
gathered per edge in edge-major layout [128e, 384] tiles via dma_gather.
Per edge tile: one DVE broadcast-multiply P = gat * eh9 (bf16), one scatter
matmul UT[64,288] += S_t.T @ P per tile against the host-built scatter matrix
(1/deg folded in), a per-window DVE reduce over c, a transpose matmul back to
[o, n] layout, root matmul + bias + relu.  Everything bf16 except PSUM
accumulation and the final output.

Sharding: edges sorted by dst; core k owns dst in [2048k, 2048(k+1)).
32 windows x 64 nodes, 5 edge-tiles (640 slots) per window, 160 tiles
= 20480 edge slots per core.  Per layer: AllGather of the per-core H9 slice.
The edge-network (eh) is computed on host (it is layer-invariant).
"""

import sys
from contextlib import ExitStack

import numpy as np
import ml_dtypes

sys.path.insert(0, "/opt/trn_rl_repo")

import concourse.bass as bass  # noqa: E402
import concourse.tile as tile  # noqa: E402
from concourse import bacc, mybir  # noqa: E402
from concourse.bass_utils import run_bass_kernel_spmd  # noqa: E402

N = 16384
E = 131072
W = 32
DEPTH = 4
NCORES = 8
NPC = N // NCORES          # 2048 nodes per core
WIN = 64                   # nodes per scatter window
TPW = 5                    # edge tiles per window (640 slots)
WPC = NPC // WIN           # 32 windows per core
NT = WPC * TPW             # 160 edge tiles per core
EPC = NT * 128             # 20480 edge slots per core
GRP = 512                  # nodes per group
NGRP = NPC // GRP          # 4
TPG = NT // NGRP           # 40 tiles per group
EPG = TPG * 128            # 5120 edge slots per group
H9W = W * 9                # 288 payload columns of H9
STEP = 384                 # padded H9 row (768 bytes, 256B-aligned)
F32 = mybir.dt.float32
BF16 = mybir.dt.bfloat16
I16 = mybir.dt.int16
AF = mybir.ActivationFunctionType
ALU = mybir.AluOpType
BF = ml_dtypes.bfloat16


def build_program(fc2b_val=0.0, fake_collective=False):
    nc = bacc.Bacc("TRN2", target_bir_lowering=False, debug=False,
                   num_devices=NCORES)

    d_xtk = nc.declare_dram_parameter("xtk", [4, NPC], BF16, isOutput=False)
    d_fc1 = nc.declare_dram_parameter("fc1a", [4, W], BF16, isOutput=False)
    d_k2 = nc.declare_dram_parameter("k2pr", [W, H9W], BF16, isOutput=False)
    d_root = nc.declare_dram_parameter("rootw", [W, W], BF16, isOutput=False)
    d_cb = nc.declare_dram_parameter("cbias", [W, 1], F32, isOutput=False)
    d_fc2 = nc.declare_dram_parameter("fc2w", [W, 1], BF16, isOutput=False)
    d_eh = nc.declare_dram_parameter("eh9b", [128, NT * 9], BF16, isOutput=False)
    d_s = nc.declare_dram_parameter("stil", [128, NT * WIN], BF16, isOutput=False)
    d_gidx = nc.declare_dram_parameter("gidx", [128, EPC // 16], I16, isOutput=False)
    d_id = nc.declare_dram_parameter("ident", [128, 128], BF16, isOutput=False)
    d_out = nc.declare_dram_parameter("out", [1, NPC], F32, isOutput=True)

    cc_in = nc.dram_tensor("cc_in", [NPC, STEP], BF16)
    cc_out_a = nc.dram_tensor("cc_out_a", [N, STEP], BF16, addr_space="Shared")
    cc_out_b = nc.dram_tensor("cc_out_b", [N, STEP], BF16, addr_space="Shared")

    with ExitStack() as ctx:
        tc = ctx.enter_context(tile.TileContext(nc))

        persist = ctx.enter_context(tc.tile_pool(name="persist", bufs=1))
        hown = persist.tile([W, NPC], BF16, tag="hown")
        s_sb = persist.tile([128, NT * WIN], BF16, tag="s_sb")
        eh9b = persist.tile([128, NT * 9], BF16, tag="eh9b")
        gidx = persist.tile([128, EPC // 16], I16, tag="gidx")
        k2pr = persist.tile([W, H9W], BF16, tag="k2pr")
        fc1a = persist.tile([4, W], BF16, tag="fc1a")
        rootw = persist.tile([W, W], BF16, tag="rootw")
        cbias = persist.tile([W, 1], F32, tag="cbias")
        fc2w = persist.tile([W, 1], BF16, tag="fc2w")
        ident = persist.tile([128, 128], BF16, tag="ident")
        outT = persist.tile([1, NPC], F32, tag="outT")

        stage = ctx.enter_context(tc.tile_pool(name="stage", bufs=2))
        gpool = ctx.enter_context(tc.tile_pool(name="gat", bufs=2))
        ppool = ctx.enter_context(tc.tile_pool(name="p9", bufs=3))
        h9pool = ctx.enter_context(tc.tile_pool(name="h9sb", bufs=2))
        atpool = ctx.enter_context(tc.tile_pool(name="aggT", bufs=2))
        ps_ut = ctx.enter_context(tc.tile_pool(name="ut", bufs=2, space="PSUM"))
        ps_agg = ctx.enter_context(tc.tile_pool(name="agg", bufs=2, space="PSUM"))
        ps_h9 = ctx.enter_context(tc.tile_pool(name="h9", bufs=2, space="PSUM"))
        ps_sm = ctx.enter_context(tc.tile_pool(name="small", bufs=2, space="PSUM"))

        dma = nc.sync.dma_start

        dma(fc1a[:], d_fc1[:])
        dma(k2pr[:], d_k2[:])
        dma(rootw[:], d_root[:])
        dma(cbias[:], d_cb[:])
        dma(fc2w[:], d_fc2[:])
        dma(ident[:], d_id[:])
        dma(gidx[:], d_gidx[:])
        dma(eh9b[:], d_eh[:])
        hs = NT * WIN // 2
        for i in range(2):
            dma(s_sb[:, i * hs:(i + 1) * hs], d_s[:, i * hs:(i + 1) * hs])

        # node encoder for own slice: hown = (x @ fc1_w.T + fc1_b).T
        xtk = stage.tile([4, NPC], BF16, tag="stg")
        dma(xtk[:], d_xtk[:])
        for q in range(NGRP):
            ps = ps_sm.tile([W, GRP], F32, tag="sm")
            nc.tensor.matmul(ps[:], lhsT=fc1a[:], rhs=xtk[:, q * GRP:(q + 1) * GRP],
                             start=True, stop=True)
            nc.scalar.activation(hown[:, q * GRP:(q + 1) * GRP], ps[:], AF.Copy)

        ccouts = [cc_out_a, cc_out_b]

        def ship(g):
            """Project H9 for hown group g and DMA it to cc_in rows."""
            h9t = h9pool.tile([128, NGRP, H9W], BF16, name="h9t", tag="h9t")
            for m in range(NGRP):
                nt = g * NGRP + m
                ph = ps_h9.tile([128, H9W], F32, name="ph")
                nc.tensor.matmul(ph[:], lhsT=hown[:, nt * 128:(nt + 1) * 128],
                                 rhs=k2pr[:], start=True, stop=True)
                nc.scalar.activation(h9t[:, m, :], ph[:], AF.Copy)
            dma(cc_in[g * GRP:(g + 1) * GRP, 0:H9W]
                .rearrange('(t p) e -> p t e', p=128), h9t[:])

        def ag_half(h, parity):
            """AllGather one 1024-row half of cc_in into the parity table."""
            cco = ccouts[parity]
            HN = NPC // 2
            if fake_collective:
                dma(cco[h * HN:(h + 1) * HN, :], cc_in[h * HN:(h + 1) * HN, :])
            else:
                nc.gpsimd.collective_compute(
                    "AllGather", ALU.bypass,
                    replica_groups=[list(range(NCORES))],
                    ins=[cc_in[h * HN:(h + 1) * HN, :].opt()],
                    outs=[cco[h * (N // 2):(h + 1) * (N // 2), :].opt()],
                )

        for g in range(NGRP):
            ship(g)
        ag_half(0, 0)
        ag_half(1, 0)

        for layer in range(DEPTH):
            cco = ccouts[layer % 2]
            for g in range(NGRP):
                if layer < DEPTH - 1 and g == NGRP - 1:
                    ag_half(0, (layer + 1) % 2)
                gat = gpool.tile([128, TPG, STEP], BF16, name="gat", tag="gat")
                for wg in range(WPC // NGRP):   # one 640-idx gather per window
                    w = g * (WPC // NGRP) + wg
                    nc.gpsimd.dma_gather(
                        gat[:, wg * TPW:(wg + 1) * TPW, :], cco[:],
                        gidx[:, w * (TPW * 8):(w + 1) * (TPW * 8)],
                        num_idxs=TPW * 128, num_idxs_reg=TPW * 128,
                        elem_size=STEP,
                    )
                agg = ps_agg.tile([W, GRP], F32, name="agg")
                nc.tensor.matmul(agg[:], lhsT=rootw[:],
                                 rhs=hown[:, g * GRP:(g + 1) * GRP],
                                 start=True, stop=False)
                atile = atpool.tile([128, NGRP, W], BF16, name="aggT", tag="aggT")
                for wg in range(WPC // NGRP):       # 8 windows in group
                    w = g * (WPC // NGRP) + wg
                    t0 = w * TPW
                    j0 = t0 - g * TPG
                    base = 64 * (wg % 2)
                    p9t = ppool.tile([128, TPW, H9W], BF16, tag="p9t")
                    ehv = (eh9b[:, t0 * 9:(t0 + TPW) * 9]
                           .rearrange('p (t c) -> p t c', c=9)
                           .rearrange('p t (x c) -> p t x c', x=1)
                           .broadcast_to([128, TPW, W, 9]))
                    nc.vector.tensor_tensor(
                        p9t[:].rearrange('p t (o c) -> p t o c', c=9),
                        gat[:, j0:j0 + TPW, 0:H9W].rearrange('p t (o c) -> p t o c', c=9),
                        ehv, op=ALU.mult)
                    ut = ps_ut.tile([128, H9W], F32, name="ut")
                    for t5 in range(TPW):
                        t = t0 + t5
                        nc.tensor.matmul(
                            ut[base:base + 64, :],
                            lhsT=s_sb[:, t * WIN:(t + 1) * WIN],
                            rhs=p9t[:, t5, :],
                            start=(t5 == 0), stop=(t5 == TPW - 1),
                            tile_position=(0, base))
                    m = wg // 2
                    with nc.allow_low_precision(reason="bf16 agg feeds bf16 matmuls"):
                        nc.vector.reduce_sum(
                            atile[base:base + 64, m, :],
                            ut[base:base + 64, :].rearrange('p (o c) -> p o c', c=9),
                            axis=mybir.AxisListType.X)
                for m in range(NGRP):
                    nc.tensor.matmul(agg[:, m * 128:(m + 1) * 128],
                                     lhsT=atile[:, m, :], rhs=ident[:],
                                     start=False, stop=(m == NGRP - 1))
                nc.scalar.activation(hown[:, g * GRP:(g + 1) * GRP], agg[:],
                                     AF.Relu, bias=cbias[:])
                if layer < DEPTH - 1:
                    ship(g)
            if layer < DEPTH - 1:
                ag_half(1, (layer + 1) % 2)

        for q in range(NGRP):
            ps = ps_sm.tile([1, GRP], F32, tag="sm")
            nc.tensor.matmul(ps[:], lhsT=fc2w[:], rhs=hown[:, q * GRP:(q + 1) * GRP],
                             start=True, stop=True)
            nc.scalar.activation(outT[:, q * GRP:(q + 1) * GRP], ps[:], AF.Copy,
                                 bias=float(fc2b_val))
        dma(d_out[:, :], outT[:])

    nc.finalize()
    return nc


def _prep_inputs(inputs):
    x = np.asarray(inputs["x"], np.float32)
    ei = np.asarray(inputs["edge_index"])
    ea = np.asarray(inputs["edge_attr"], np.float32)
    fc1_w = np.asarray(inputs["fc1_w"], np.float32)
    fc1_b = np.asarray(inputs["fc1_b"], np.float32)
    k1_w = np.asarray(inputs["k1_w"], np.float32)
    k1_b = np.asarray(inputs["k1_b"], np.float32)
    k2_w = np.asarray(inputs["k2_w"], np.float32)
    k2_b = np.asarray(inputs["k2_b"], np.float32)
    root_w = np.asarray(inputs["root_w"], np.float32)
    conv_bias = np.asarray(inputs["conv_bias"], np.float32)
    fc2_w = np.asarray(inputs["fc2_w"], np.float32)
    fc2_b = np.asarray(inputs["fc2_b"], np.float32)

    src = ei[0].astype(np.int64)
    dst = ei[1].astype(np.int64)
    deg = np.clip(np.bincount(dst, minlength=N).astype(np.float32), 1.0, None)

    # edge network on host (layer-invariant): eh9 = [relu(ea @ k1.T + b), 1]
    eh = np.maximum(ea @ k1_w.T + k1_b, 0.0)            # [E, 8]
    eh9_full = np.concatenate([eh, np.ones((E, 1), np.float32)], axis=1)

    xt = np.concatenate([x.T, np.ones((1, N), np.float32)], axis=0)  # [4, N]
    fc1a = np.concatenate([fc1_w.T, fc1_b[None, :]], axis=0)
    # k2pr[i, o*9+c] = M_c[i, o]; M_c[i,o] = k2_w[i*W+o, c] (c<8), k2_b[i*W+o] (c=8)
    k2pr = np.concatenate(
        [k2_w.reshape(W, W, 8), k2_b.reshape(W, W, 1)], axis=2
    ).reshape(W, H9W)

    shared = {
        "fc1a": fc1a.astype(BF),
        "k2pr": k2pr.astype(BF),
        "rootw": root_w.astype(BF),
        "cbias": np.ascontiguousarray(conv_bias[:, None]).astype(np.float32),
        "fc2w": np.ascontiguousarray(fc2_w.T).astype(BF),
        "ident": np.eye(128, dtype=np.float32).astype(BF),
    }

    in_maps = []
    jarr = np.arange(EPC)
    for k in range(NCORES):
        m = (dst >= k * NPC) & (dst < (k + 1) * NPC)
        es, ed, ehk = src[m], dst[m], eh9_full[m]
        local = ed - k * NPC
        win = local // WIN
        jloc = local % WIN

        gsrc = np.zeros(EPC, np.int64)
        sval = np.zeros(EPC, np.float32)
        jl = np.zeros(EPC, np.int64)
        eh9s = np.zeros((EPC, 9), np.float32)
        for w in range(WPC):
            idxs = np.flatnonzero(win == w)
            assert len(idxs) <= TPW * 128, f"window overflow: {len(idxs)}"
            base = w * TPW * 128
            sl = slice(base, base + len(idxs))
            gsrc[sl] = es[idxs]
            sval[sl] = 1.0 / deg[ed[idxs]]
            jl[sl] = jloc[idxs]
            eh9s[sl] = ehk[idxs]

        S = np.zeros((NT, 128, WIN), np.float32)
        S[jarr // 128, jarr % 128, jl] = sval
        stil = np.ascontiguousarray(S.transpose(1, 0, 2).reshape(128, NT * WIN))

        eh9b = np.ascontiguousarray(
            eh9s.reshape(NT, 128, 9).transpose(1, 0, 2).reshape(128, NT * 9))

        # table row for node n = r*2048+j: first halves of every rank's
        # slice land in rows [0, 8192), second halves in [8192, 16384)
        rr, jj = gsrc // NPC, gsrc % NPC
        grow = np.where(jj < NPC // 2,
                        rr * (NPC // 2) + jj,
                        N // 2 + rr * (NPC // 2) + (jj - NPC // 2))
        gx = np.zeros((128, EPC // 16), np.int16)
        i = np.arange(EPC)
        for a in range(8):
            gx[16 * a + i % 16, i // 16] = grow

        in_maps.append({
            **shared,
            "xtk": np.ascontiguousarray(xt[:, k * NPC:(k + 1) * NPC]).astype(BF),
            "eh9b": eh9b.astype(BF),
            "stil": stil.astype(BF),
            "gidx": gx,
        })
    return in_maps


def _run(inputs, trace=False, fake_collective=False, **kw):
    in_maps = _prep_inputs(inputs)
    fc2_b = np.asarray(inputs["fc2_b"], np.float32)
    nc = build_program(float(fc2_b[0]), fake_collective=fake_collective)
    res = run_bass_kernel_spmd(nc, in_maps, core_ids=list(range(NCORES)),
                               trace=trace, **kw)
    outs = [res.results[k]["out"].reshape(1, NPC) for k in range(NCORES)]
    full = np.concatenate(outs, axis=1).T.astype(np.float32)
    return full, res


def kernel(**inputs) -> np.ndarray:
    return _run(inputs, trace=False)[0]
